# revision 3
# baseline (speedup 1.0000x reference)
"""Trainium2 Bass kernel for the AttentionBlock problem.

Math (per batch b):
  x_down = avgpool4x4(x)            # [C, 32, 32] -> xf [C, N], N=1024
  q,k = Wq/Wk @ xf + b              # [8, N]
  v = Wv @ xf + bv                  # [C, N]
  attn = softmax_n(q^T k)           # [N, N]
  out[c,m] = sum_n v[c,n] attn[m,n]
  y = gamma * upsample_bilinear(out) + x

Mapping (one NeuronCore per batch, 8 cores):
  - x resident in SBUF as two [128, 16384] tiles (c-halves); pooled via DVE
    (w-reduce + h pair adds). The 1/16 mean factor is folded into the weights.
  - q,k computed together (lhsT = [WqT|WkT], M=16), f32r matmuls.
  - logits computed transposed: Lt[n, m] = k^T q, n on partitions -> exp on ACT
    -> Et bf16. Vt[n, c] computed directly (lhsT = xf chunk), ones column
    appended for the softmax denominator.
  - attn@V: O[m, c] = Et^T Vt accumulated over n-chunks; denominator lands in
    column 256; normalize via per-partition reciprocal * tensor_scalar.
  - Upsample fused into one sparse matmul: y[c, (H,W)-slice] = sum_m O[m, c] *
    slab[m, slice], slab[m, (H,W)] = gamma*U[H, hb(m)]*U[W, wb(m)] generated
    on-device (4 ACT ops per slab). Residual add fused into the PSUM->SBUF
    copy (DVE tensor_add with x), written back in place over x, then DMA out.
"""

import numpy as np

B, C, H, W = 8, 256, 128, 128
HD, WD = 32, 32
N = HD * WD  # 1024
CQ = 8
NCORES = 8

_CACHE = {}


def _resize_matrix(dst: int, src: int) -> np.ndarray:
    """Bilinear (half-pixel, edge-renormalized) resize matrix, matches
    jax.image.resize(method='linear') for upsampling."""
    scale = dst / src
    pos = (np.arange(dst, dtype=np.float64) + 0.5) / scale - 0.5
    j = np.arange(src, dtype=np.float64)
    w = np.maximum(0.0, 1.0 - np.abs(pos[:, None] - j[None, :]))
    w = w / w.sum(axis=1, keepdims=True)
    return w.astype(np.float32)  # [dst, src]


def _build_bass():
    import concourse.bass as bass
    import concourse.tile as tile
    from concourse import bacc, mybir

    f32 = mybir.dt.float32
    f32r = mybir.dt.float32r
    bf16 = mybir.dt.bfloat16
    AF = mybir.ActivationFunctionType
    AX = mybir.AxisListType
    AL = mybir.AluOpType

    nc = bacc.Bacc("TRN2", target_bir_lowering=False, debug=False)

    x_d = nc.dram_tensor("x", [C, H * W], f32, kind="ExternalInput")
    wqk_d = nc.dram_tensor("wqk", [C, 40], bf16, kind="ExternalInput")
    bqk_d = nc.dram_tensor("bqk", [1, 40], bf16, kind="ExternalInput")
    wv_d = nc.dram_tensor("wv", [C, C], bf16, kind="ExternalInput")
    bv_d = nc.dram_tensor("bv", [1, C], bf16, kind="ExternalInput")
    gam_d = nc.dram_tensor("gamma", [1, 1], f32, kind="ExternalInput")
    amat_d = nc.dram_tensor("amat", [128, 8 * H], f32, kind="ExternalInput")
    bmat_d = nc.dram_tensor("bmat", [128, W], f32, kind="ExternalInput")
    y_d = nc.dram_tensor("y", [C, H * W], f32, kind="ExternalOutput")

    with tile.TileContext(nc) as tc:
        with (
            tc.tile_pool(name="xbig", bufs=1) as xbig,
            tc.tile_pool(name="persist", bufs=1) as persist,
        ):
            x0 = xbig.tile([128, H * W], f32)
            x1 = xbig.tile([128, H * W], f32)
            xt = [x0, x1]

            # persistent tensors
            et_sb = persist.tile([128, 8, N], bf16)      # Et[n-chunk][n_l, m]
            vt_sb = persist.tile([128, 8, C + 1], bf16)  # Vt[n-chunk][n_l, c|1]
            o_sb = persist.tile([128, 8, C], bf16)        # O[m-chunk][m_l, c]
            rec_sb = persist.tile([128, 8], f32)
            a_sb = persist.tile([128, 8 * H], f32)
            b_sb = persist.tile([128, W], f32)
            bg_sb = persist.tile([128, W], f32)
            gam_sb = persist.tile([128, 1], f32)
            wqk_sb = persist.tile([128, 2, 40], bf16)
            bqk_sb = persist.tile([1, 40], bf16)
            wv_sb = persist.tile([128, 2, C], bf16)
            bv_sb = persist.tile([1, C], bf16)
            q_sb = persist.tile([CQ, N], bf16)
            k_sb = persist.tile([CQ, N], bf16)

            # constant/weight DMAs
            nc.sync.dma_start(out=wqk_sb[:],
                              in_=wqk_d[:].rearrange("(t p) o -> p t o", p=128))
            nc.sync.dma_start(out=bqk_sb[:], in_=bqk_d[:])
            nc.sync.dma_start(out=wv_sb[:],
                              in_=wv_d[:].rearrange("(t p) o -> p t o", p=128))
            nc.sync.dma_start(out=bv_sb[:], in_=bv_d[:])
            nc.sync.dma_start(out=a_sb[:], in_=amat_d[:])
            nc.sync.dma_start(out=b_sb[:], in_=bmat_d[:])
            nc.sync.dma_start(out=gam_sb[:], in_=gam_d[:].to_broadcast((128, 1)))
            nc.vector.tensor_scalar_mul(bg_sb[:], b_sb[:], gam_sb[:, 0:1])

            # x input DMAs: 4 chunks of 32 h-rows per c-half (2 MB each)
            for t in range(2):
                for hc in range(4):
                    sl = bass.ds(hc * 4096, 4096)
                    nc.sync.dma_start(out=xt[t][:, sl],
                                      in_=x_d[t * 128:(t + 1) * 128, sl])

            with (
                tc.tile_pool(name="phase1", bufs=1) as ph1,
                tc.tile_pool(name="ptmp", bufs=3) as ptmp,
                tc.tile_pool(name="ps_qk", bufs=1, space="PSUM") as ps_qk,
                tc.tile_pool(name="ps_lt", bufs=2, space="PSUM") as ps_lt,
                tc.tile_pool(name="ps_vt", bufs=2, space="PSUM") as ps_vt,
            ):
                xf_sb = ph1.tile([128, 2, N], bf16)
                ones_sb = ph1.tile([1, N], bf16)
                nc.vector.memset(ones_sb[:], 1.0)

                # ---- pooling: strips of 16 h rows (4 hb rows) ----
                for t in range(2):
                    for st in range(8):
                        strip = xt[t][:, bass.ds(st * 2048, 2048)].rearrange(
                            "p (h wb dw) -> p h wb dw", h=16, wb=32)
                        rsum = ptmp.tile([128, 16, 32], f32, tag="rsum")
                        nc.vector.tensor_reduce(
                            out=rsum[:], in_=strip, axis=AX.X, op=AL.add)
                        r2 = rsum[:].rearrange("p (h two) wb -> p h two wb", two=2)
                        t1 = ptmp.tile([128, 8, 32], f32, tag="t1")
                        nc.vector.tensor_add(t1[:], r2[:, :, 0, :], r2[:, :, 1, :])
                        t2 = t1[:].rearrange("p (h two) wb -> p h two wb", two=2)
                        xfs = xf_sb[:, t, bass.ds(st * 128, 128)].rearrange(
                            "p (hb wb) -> p hb wb", hb=4)
                        nc.vector.tensor_add(xfs, t2[:, :, 0, :], t2[:, :, 1, :])

                # ---- q, k (combined M=16) ----
                qk_ps = ps_qk.tile([40, N], f32)
                for ms in range(2):
                    sl = bass.ds(ms * 512, 512)
                    nc.tensor.matmul(qk_ps[:, sl], wqk_sb[:, 0, :],
                                     xf_sb[:, 0, sl], start=True, stop=False)
                    nc.tensor.matmul(qk_ps[:, sl], wqk_sb[:, 1, :],
                                     xf_sb[:, 1, sl], start=False, stop=False)
                    nc.tensor.matmul(qk_ps[:, sl], bqk_sb[:],
                                     ones_sb[:, sl], start=False, stop=True)
                nc.scalar.copy(q_sb[:], qk_ps[0:CQ, :])
                nc.scalar.copy(k_sb[:], qk_ps[32:40, :])

                # ---- Vt chunks ----
                nc.vector.memset(vt_sb[:, :, C:C + 1], 1.0)
                for nk in range(8):
                    nsl = bass.ds(nk * 128, 128)
                    vt_ps = ps_vt.tile([128, C], f32, tag="vt")
                    nc.tensor.matmul(vt_ps[:], xf_sb[:, 0, nsl],
                                     wv_sb[:, 0, :], start=True, stop=False)
                    nc.tensor.matmul(vt_ps[:], xf_sb[:, 1, nsl],
                                     wv_sb[:, 1, :], start=False, stop=False)
                    nc.tensor.matmul(vt_ps[:], ones_sb[:, nsl],
                                     bv_sb[:], start=False, stop=True)
                    nc.scalar.copy(vt_sb[:, nk, 0:C], vt_ps[:])

                # ---- logits (transposed) + exp ----
                for nk in range(8):
                    nsl = bass.ds(nk * 128, 128)
                    lt_ps = ps_lt.tile([128, N], f32, tag="lt")
                    for ms in range(2):
                        sl = bass.ds(ms * 512, 512)
                        nc.tensor.matmul(lt_ps[:, sl], k_sb[:, nsl],
                                         q_sb[:, sl], start=True, stop=True)
                    nc.scalar.activation(et_sb[:, nk, :], lt_ps[:], func=AF.Exp)

            # ---- attn @ V ----
            with tc.tile_pool(name="ps_o", bufs=3, space="PSUM") as ps_o:
                for mk in range(8):
                    msl = bass.ds(mk * 128, 128)
                    o_ps = ps_o.tile([128, C + 1], f32, tag="o")
                    for nk in range(8):
                        nc.tensor.matmul(o_ps[:], et_sb[:, nk, msl], vt_sb[:, nk, :],
                                         start=(nk == 0), stop=(nk == 7))
                    nc.vector.reciprocal(rec_sb[:, mk:mk + 1], o_ps[:, C:C + 1])
                    nc.vector.tensor_scalar_mul(o_sb[:, mk, :], o_ps[:, 0:C],
                                                rec_sb[:, mk:mk + 1])

            # ---- fused upsample + residual + output ----
            with (
                tc.tile_pool(name="slabs", bufs=3) as slabs,
                tc.tile_pool(name="ps_y", bufs=4, space="PSUM") as ps_y,
            ):
                for s in range(32):
                    hbs = sorted({hb for hb in (s - 1, s, s + 1) if 0 <= hb < 32})
                    ks = sorted({hb // 4 for hb in hbs})
                    slab_tiles = []
                    for kc in ks:
                        slab = slabs.tile([128, 4, W], bf16, tag="slab")
                        for hl in range(4):
                            col = kc * H + 4 * s + hl
                            nc.scalar.mul(slab[:, hl, :], bg_sb[:],
                                          mul=a_sb[:, col:col + 1])
                        slab_tiles.append(slab)
                    for ch in range(2):
                        y_ps = ps_y.tile([128, 512], f32, tag="y")
                        for i, kc in enumerate(ks):
                            nc.tensor.matmul(
                                y_ps[:], o_sb[:, kc, ch * 128:(ch + 1) * 128],
                                slab_tiles[i][:],
                                start=(i == 0), stop=(i == len(ks) - 1))
                        osl = bass.ds(s * 512, 512)
                        nc.vector.tensor_add(xt[ch][:, osl], y_ps[:], xt[ch][:, osl])
                    if s % 8 == 7:
                        g = s // 8
                        gsl = bass.ds(g * 4096, 4096)
                        for ch in range(2):
                            nc.sync.dma_start(out=y_d[ch * 128:(ch + 1) * 128, gsl],
                                              in_=xt[ch][:, gsl])
    nc.compile()
    return nc


def _get_nc():
    if "nc" not in _CACHE:
        _CACHE["nc"] = _build_bass()
    return _CACHE["nc"]


def kernel(x, Wq, bq, Wk, bk, Wv, bv, gamma):
    from concourse.bass_utils import run_bass_kernel_spmd

    x = np.ascontiguousarray(np.asarray(x, dtype=np.float32))
    U = _resize_matrix(H, HD)  # [128, 32]

    p = np.arange(128)
    amat = np.zeros((128, 8 * H), dtype=np.float32)
    for kc in range(8):
        amat[:, kc * H:(kc + 1) * H] = U[:, 4 * kc + p // 32].T
    bmat = np.ascontiguousarray(U[:, p % 32].T)  # [128, W] -> B[p, w] = U[w, p%32]

    import ml_dtypes
    bfd = ml_dtypes.bfloat16
    wqk = np.zeros((C, 40), dtype=bfd)
    wqk[:, 0:8] = (np.asarray(Wq).T / 16.0).astype(bfd)
    wqk[:, 32:40] = (np.asarray(Wk).T / 16.0).astype(bfd)
    bqk = np.zeros((1, 40), dtype=bfd)
    bqk[0, 0:8] = np.asarray(bq).astype(bfd)
    bqk[0, 32:40] = np.asarray(bk).astype(bfd)
    wv = np.ascontiguousarray(np.asarray(Wv).T / 16.0).astype(bfd)
    bvr = np.asarray(bv)[None, :].astype(bfd)
    gam = np.asarray(gamma).reshape(1, 1).astype(np.float32)

    nc = _get_nc()
    in_maps = []
    for i in range(NCORES):
        in_maps.append({
            "x": np.ascontiguousarray(x[i].reshape(C, H * W)),
            "wqk": wqk, "bqk": bqk, "wv": wv, "bv": bvr,
            "gamma": gam, "amat": amat, "bmat": bmat,
        })
    res = run_bass_kernel_spmd(nc, in_maps, core_ids=list(range(NCORES)))
    y = np.stack([r["y"].reshape(C, H, W) for r in res.results])
    return y.astype(np.float32)


if __name__ == "__main__":
    rng = np.random.default_rng(0)
    inputs = {
        "x": rng.standard_normal((B, C, H, W), dtype=np.float32),
        "Wq": rng.standard_normal((CQ, C), dtype=np.float32) * 0.05,
        "bq": rng.standard_normal((CQ,), dtype=np.float32) * 0.05,
        "Wk": rng.standard_normal((CQ, C), dtype=np.float32) * 0.05,
        "bk": rng.standard_normal((CQ,), dtype=np.float32) * 0.05,
        "Wv": rng.standard_normal((C, C), dtype=np.float32) * 0.05,
        "bv": rng.standard_normal((C,), dtype=np.float32) * 0.05,
        "gamma": np.zeros((1,), dtype=np.float32),
    }
    y = kernel(**inputs)
    print("out", y.shape, y.dtype, float(np.abs(y - inputs["x"]).max()))


# revision 13
# speedup vs baseline: 331.2443x; 331.2443x over previous
"""Trainium2 Bass kernel for the AttentionBlock problem.

Math (per batch b):
  x_down = avgpool4x4(x)            # [C, 32, 32] -> xf [C, N], N=1024
  q,k = Wq/Wk @ xf + b              # [8, N]
  v = Wv @ xf + bv                  # [C, N]
  attn = softmax_n(q^T k)           # [N, N]
  out[c,m] = sum_n v[c,n] attn[m,n]
  y = gamma * upsample_bilinear(out) + x

Mapping (one NeuronCore per batch, 8 cores):
  - x resident in SBUF as two [128, 16384] tiles (c-halves); pooled via DVE
    (w-reduce + h pair adds). The 1/16 mean factor is folded into the weights.
  - q,k computed together (lhsT = [WqT|WkT], M=16), f32r matmuls.
  - logits computed transposed: Lt[n, m] = k^T q, n on partitions -> exp on ACT
    -> Et bf16. Vt[n, c] computed directly (lhsT = xf chunk), ones column
    appended for the softmax denominator.
  - attn@V: O[m, c] = Et^T Vt accumulated over n-chunks; denominator lands in
    column 256; normalize via per-partition reciprocal * tensor_scalar.
  - Upsample fused into one sparse matmul: y[c, (H,W)-slice] = sum_m O[m, c] *
    slab[m, slice], slab[m, (H,W)] = gamma*U[H, hb(m)]*U[W, wb(m)] generated
    on-device (4 ACT ops per slab). Residual add fused into the PSUM->SBUF
    copy (DVE tensor_add with x), written back in place over x, then DMA out.
"""

import numpy as np

B, C, H, W = 8, 256, 128, 128
HD, WD = 32, 32
N = HD * WD  # 1024
CQ = 8
NCORES = 8

_CACHE = {}


def _resize_matrix(dst: int, src: int) -> np.ndarray:
    """Bilinear (half-pixel, edge-renormalized) resize matrix, matches
    jax.image.resize(method='linear') for upsampling."""
    scale = dst / src
    pos = (np.arange(dst, dtype=np.float64) + 0.5) / scale - 0.5
    j = np.arange(src, dtype=np.float64)
    w = np.maximum(0.0, 1.0 - np.abs(pos[:, None] - j[None, :]))
    w = w / w.sum(axis=1, keepdims=True)
    return w.astype(np.float32)  # [dst, src]


def _build_bass():
    import concourse.bass as bass
    import concourse.tile as tile
    from concourse import bacc, mybir

    f32 = mybir.dt.float32
    f32r = mybir.dt.float32r
    bf16 = mybir.dt.bfloat16
    AF = mybir.ActivationFunctionType
    AX = mybir.AxisListType
    AL = mybir.AluOpType

    nc = bacc.Bacc("TRN2", target_bir_lowering=False, debug=False)

    x_d = nc.dram_tensor("x", [C, H * W], f32, kind="ExternalInput")
    wqk_d = nc.dram_tensor("wqk", [C, 40], bf16, kind="ExternalInput")
    bqk_d = nc.dram_tensor("bqk", [1, 40], bf16, kind="ExternalInput")
    wv_d = nc.dram_tensor("wv", [C, C], bf16, kind="ExternalInput")
    bv_d = nc.dram_tensor("bv", [1, C], bf16, kind="ExternalInput")
    gam_d = nc.dram_tensor("gamma", [1, 1], f32, kind="ExternalInput")
    amat_d = nc.dram_tensor("amat", [128, 8 * H], f32, kind="ExternalInput")
    bmat_d = nc.dram_tensor("bmat", [128, W], f32, kind="ExternalInput")
    y_d = nc.dram_tensor("y", [C, H * W], f32, kind="ExternalOutput")

    with tile.TileContext(nc) as tc:
        with (
            tc.tile_pool(name="xbig", bufs=1) as xbig,
            tc.tile_pool(name="persist", bufs=1) as persist,
        ):
            x0 = xbig.tile([128, H * W], f32)
            x1 = xbig.tile([128, H * W], f32)
            xt = [x0, x1]

            # persistent tensors
            et_sb = persist.tile([128, 8, N], bf16)      # Et[n-chunk][n_l, m]
            vt_sb = persist.tile([128, 8, C + 1], bf16)  # Vt[n-chunk][n_l, c|1]
            o_sb = persist.tile([128, 8, C], bf16)        # O[m-chunk][m_l, c]
            rec_sb = persist.tile([128, 8], f32)
            a_sb = persist.tile([128, 8 * H], f32)
            b_sb = persist.tile([128, W], f32)
            gam_sb = persist.tile([128, 1], f32)
            wqk_sb = persist.tile([128, 2, 40], bf16)
            bqk_sb = persist.tile([1, 40], bf16)
            wv_sb = persist.tile([128, 2, C], bf16)
            bv_sb = persist.tile([1, C], bf16)
            q_sb = persist.tile([CQ, N], bf16)
            k_sb = persist.tile([CQ, N], bf16)


            arena_pool_cm = tc.tile_pool(name="arena", bufs=3)
            arenas = arena_pool_cm.__enter__()
            arena = {}

            def gen_arena(kc, eng):
                r0 = max(0, 16 * kc - 4)
                r1 = min(128, 16 * kc + 20)
                cnt = r1 - r0
                t_ = arenas.tile([128, 24, W], bf16, tag="arena")
                bb = b_sb[:]
                b_bc = bass.AP(tensor=bb.tensor, offset=bb.offset,
                               ap=[bb.ap[0], [0, cnt], bb.ap[1]])
                aa = a_sb[:, kc * H + r0:kc * H + r1]
                a_bc = bass.AP(tensor=aa.tensor, offset=aa.offset,
                               ap=[aa.ap[0], aa.ap[1], [0, W]])
                eng.tensor_mul(t_[:, 0:cnt, :], b_bc, a_bc)
                arena[kc] = (t_, r0)

            for kc in range(3):
                gen_arena(kc, nc.vector)

            # x input DMAs: strip-interleaved (16 h-rows x both c-halves)
            for st in range(8):
                for t in range(2):
                    sl = bass.ds(st * 2048, 2048)
                    nc.sync.dma_start(out=xt[t][:, sl],
                                      in_=x_d[t * 128:(t + 1) * 128, sl])

            with (
                tc.tile_pool(name="phase1", bufs=1) as ph1,
                tc.tile_pool(name="ptmp", bufs=1) as ptmp,
                tc.tile_pool(name="ps_qk", bufs=1, space="PSUM") as ps_qk,
                tc.tile_pool(name="ps_lt", bufs=4, space="PSUM") as ps_lt,
                tc.tile_pool(name="ps_vt", bufs=2, space="PSUM") as ps_vt,
            ):
                xf_sb = ph1.tile([128, 2, N], bf16)
                ones_sb = ph1.tile([1, N], bf16)
                nc.vector.memset(ones_sb[:], 1.0)

                # ---- pooling: strips of 16 h rows; p1 on gpsimd ----
                for st in range(8):
                    for t in range(2):
                        strip = xt[t][:, bass.ds(st * 2048, 2048)]
                        v1 = strip.rearrange("p (h two w) -> p h two w",
                                             two=2, w=128)
                        t1 = ptmp.tile([128, 8, 128], bf16, tag=f"t1_{t}")
                        p1eng = nc.gpsimd if t == 0 else nc.vector
                        p1eng.tensor_add(t1[:], v1[:, :, 0, :], v1[:, :, 1, :])
                        v2 = t1[:].rearrange("p (h two) w -> p h two w", two=2)
                        t2 = ptmp.tile([128, 4, 128], bf16, tag="t2")
                        nc.vector.tensor_add(t2[:], v2[:, :, 0, :], v2[:, :, 1, :])
                        v3 = t2[:].rearrange("p hb (wp two) -> p hb wp two",
                                             two=2)
                        t3 = ptmp.tile([128, 4, 64], bf16, tag="t3")
                        nc.vector.tensor_add(t3[:], v3[:, :, :, 0], v3[:, :, :, 1])
                        v4 = t3[:].rearrange("p hb (wb two) -> p hb wb two", two=2)
                        xfs = xf_sb[:, t, bass.ds(st * 128, 128)].rearrange(
                            "p (hb wb) -> p hb wb", hb=4)
                        nc.vector.tensor_add(xfs, v4[:, :, :, 0], v4[:, :, :, 1])

                # ---- q, k (combined M=40, k at partition 32) ----
                qk_ps = ps_qk.tile([40, N], f32)
                for ms in range(2):
                    sl = bass.ds(ms * 512, 512)
                    nc.tensor.matmul(qk_ps[:, sl], wqk_sb[:, 0, :],
                                     xf_sb[:, 0, sl], start=True, stop=False)
                    nc.tensor.matmul(qk_ps[:, sl], wqk_sb[:, 1, :],
                                     xf_sb[:, 1, sl], start=False, stop=False)
                    nc.tensor.matmul(qk_ps[:, sl], bqk_sb[:],
                                     ones_sb[:, sl], start=False, stop=True)
                    nc.scalar.copy(q_sb[:, sl], qk_ps[0:CQ, sl])
                    nc.vector.tensor_copy(k_sb[:, sl], qk_ps[32:40, sl])

                # ---- Vt chunks ----
                nc.vector.memset(vt_sb[:, :, C:C + 1], 1.0)
                for nk in range(8):
                    nsl = bass.ds(nk * 128, 128)
                    vt_ps = ps_vt.tile([128, C], f32, tag="vt")
                    nc.tensor.matmul(vt_ps[:], xf_sb[:, 0, nsl],
                                     wv_sb[:, 0, :], start=True, stop=False)
                    nc.tensor.matmul(vt_ps[:], xf_sb[:, 1, nsl],
                                     wv_sb[:, 1, :], start=False, stop=False)
                    nc.tensor.matmul(vt_ps[:], ones_sb[:, nsl],
                                     bv_sb[:], start=False, stop=True)
                    nc.scalar.copy(vt_sb[:, nk, 0:C], vt_ps[:])

                # ---- logits (transposed) + exp, streamed per 512-half ----
                for ms in range(2):
                    for nk in range(8):
                        nsl = bass.ds(nk * 128, 128)
                        sl = bass.ds(ms * 512, 512)
                        lt_ps = ps_lt.tile([128, 512], f32, tag="lt")
                        nc.tensor.matmul(lt_ps[:], k_sb[:, nsl],
                                         q_sb[:, sl], start=True, stop=True)
                        nc.scalar.activation(et_sb[:, nk, sl], lt_ps[:],
                                             func=AF.Exp)

            # ---- attn @ V ----
            with tc.tile_pool(name="ps_o", bufs=3, space="PSUM") as ps_o:
                for mk in range(8):
                    msl = bass.ds(mk * 128, 128)
                    o_ps = ps_o.tile([128, C + 1], f32, tag="o")
                    for nk in range(8):
                        nc.tensor.matmul(o_ps[:], et_sb[:, nk, msl], vt_sb[:, nk, :],
                                         start=(nk == 0), stop=(nk == 7))
                    nc.vector.reciprocal(rec_sb[:, mk:mk + 1], o_ps[:, C:C + 1])
                    nc.vector.tensor_scalar(
                        out=o_sb[:, mk, :], in0=o_ps[:, 0:C],
                        scalar1=rec_sb[:, mk:mk + 1], scalar2=gam_sb[:, 0:1],
                        op0=AL.mult, op1=AL.mult)

            # ---- fused upsample + residual + output ----
            # arena_k[p, j, w] = A[p, kH + r0 + j] * B[p, w]; the moving
            # operand for slice s, chunk k is arena_k[:, 4s-r0 : 4s-r0+4, :].
            with (
                tc.tile_pool(name="ps_y", bufs=4, space="PSUM") as ps_y,
            ):
                def slice_chunks(s):
                    hbs = {hb for hb in (s - 1, s, s + 1) if 0 <= hb < 32}
                    return sorted({hb // 4 for hb in hbs})

                for g in range(16):  # 2 slices per psum tile
                    for kc in slice_chunks(2 * g + 1):
                        if kc not in arena:
                            gen_arena(kc, nc.gpsimd)
                    for ch in range(2):
                        y_ps = ps_y.tile([128, 1024], f32, tag="y")
                        for q in range(2):
                            s = 2 * g + q
                            ks = slice_chunks(s)
                            for i, kc in enumerate(ks):
                                t_, r0 = arena[kc]
                                nc.tensor.matmul(
                                    y_ps[:, bass.ds(q * 512, 512)],
                                    o_sb[:, kc, ch * 128:(ch + 1) * 128],
                                    t_[:, 4 * s - r0:4 * s - r0 + 4, :],
                                    start=(i == 0), stop=(i == len(ks) - 1))
                        osl = bass.ds(g * 1024, 1024)
                        nc.vector.tensor_add(xt[ch][:, osl], y_ps[:], xt[ch][:, osl])
                    if g % 2 == 1:
                        gg = g // 2
                        gsl = bass.ds(gg * 2048, 2048)
                        for ch in range(2):
                            nc.sync.dma_start(out=y_d[ch * 128:(ch + 1) * 128, gsl],
                                              in_=xt[ch][:, gsl])
            arena_pool_cm.__exit__(None, None, None)
    nc.compile()
    return nc


def _get_nc():
    if "nc" not in _CACHE:
        _CACHE["nc"] = _build_bass()
    return _CACHE["nc"]


def kernel(x, Wq, bq, Wk, bk, Wv, bv, gamma):
    from concourse.bass_utils import run_bass_kernel_spmd

    x = np.ascontiguousarray(np.asarray(x, dtype=np.float32))
    U = _resize_matrix(H, HD)  # [128, 32]

    p = np.arange(128)
    amat = np.zeros((128, 8 * H), dtype=np.float32)
    for kc in range(8):
        amat[:, kc * H:(kc + 1) * H] = U[:, 4 * kc + p // 32].T
    bmat = np.ascontiguousarray(U[:, p % 32].T)  # [128, W] -> B[p, w] = U[w, p%32]

    import ml_dtypes
    bfd = ml_dtypes.bfloat16
    wqk = np.zeros((C, 40), dtype=bfd)
    wqk[:, 0:8] = (np.asarray(Wq).T / 16.0).astype(bfd)
    wqk[:, 32:40] = (np.asarray(Wk).T / 16.0).astype(bfd)
    bqk = np.zeros((1, 40), dtype=bfd)
    bqk[0, 0:8] = np.asarray(bq).astype(bfd)
    bqk[0, 32:40] = np.asarray(bk).astype(bfd)
    wv = np.ascontiguousarray(np.asarray(Wv).T / 16.0).astype(bfd)
    bvr = np.asarray(bv)[None, :].astype(bfd)
    gam = np.asarray(gamma).reshape(1, 1).astype(np.float32)

    nc = _get_nc()
    in_maps = []
    for i in range(NCORES):
        in_maps.append({
            "x": np.ascontiguousarray(x[i].reshape(C, H * W)),
            "wqk": wqk, "bqk": bqk, "wv": wv, "bv": bvr,
            "gamma": gam, "amat": amat, "bmat": bmat,
        })
    res = run_bass_kernel_spmd(nc, in_maps, core_ids=list(range(NCORES)))
    y = np.stack([r["y"].reshape(C, H, W) for r in res.results])
    return y.astype(np.float32)


if __name__ == "__main__":
    rng = np.random.default_rng(0)
    inputs = {
        "x": rng.standard_normal((B, C, H, W), dtype=np.float32),
        "Wq": rng.standard_normal((CQ, C), dtype=np.float32) * 0.05,
        "bq": rng.standard_normal((CQ,), dtype=np.float32) * 0.05,
        "Wk": rng.standard_normal((CQ, C), dtype=np.float32) * 0.05,
        "bk": rng.standard_normal((CQ,), dtype=np.float32) * 0.05,
        "Wv": rng.standard_normal((C, C), dtype=np.float32) * 0.05,
        "bv": rng.standard_normal((C,), dtype=np.float32) * 0.05,
        "gamma": np.zeros((1,), dtype=np.float32),
    }
    y = kernel(**inputs)
    print("out", y.shape, y.dtype, float(np.abs(y - inputs["x"]).max()))


# revision 16
# speedup vs baseline: 335.4711x; 1.0128x over previous
"""Trainium2 Bass kernel for the AttentionBlock problem.

Math (per batch b):
  x_down = avgpool4x4(x)            # [C, 32, 32] -> xf [C, N], N=1024
  q,k = Wq/Wk @ xf + b              # [8, N]
  v = Wv @ xf + bv                  # [C, N]
  attn = softmax_n(q^T k)           # [N, N]
  out[c,m] = sum_n v[c,n] attn[m,n]
  y = gamma * upsample_bilinear(out) + x

Mapping (one NeuronCore per batch, 8 cores):
  - x resident in SBUF as two [128, 16384] tiles (c-halves); pooled via DVE
    (w-reduce + h pair adds). The 1/16 mean factor is folded into the weights.
  - q,k computed together (lhsT = [WqT|WkT], M=16), f32r matmuls.
  - logits computed transposed: Lt[n, m] = k^T q, n on partitions -> exp on ACT
    -> Et bf16. Vt[n, c] computed directly (lhsT = xf chunk), ones column
    appended for the softmax denominator.
  - attn@V: O[m, c] = Et^T Vt accumulated over n-chunks; denominator lands in
    column 256; normalize via per-partition reciprocal * tensor_scalar.
  - Upsample fused into one sparse matmul: y[c, (H,W)-slice] = sum_m O[m, c] *
    slab[m, slice], slab[m, (H,W)] = gamma*U[H, hb(m)]*U[W, wb(m)] generated
    on-device (4 ACT ops per slab). Residual add fused into the PSUM->SBUF
    copy (DVE tensor_add with x), written back in place over x, then DMA out.
"""

import numpy as np

B, C, H, W = 8, 256, 128, 128
HD, WD = 32, 32
N = HD * WD  # 1024
CQ = 8
NCORES = 8

_CACHE = {}


def _resize_matrix(dst: int, src: int) -> np.ndarray:
    """Bilinear (half-pixel, edge-renormalized) resize matrix, matches
    jax.image.resize(method='linear') for upsampling."""
    scale = dst / src
    pos = (np.arange(dst, dtype=np.float64) + 0.5) / scale - 0.5
    j = np.arange(src, dtype=np.float64)
    w = np.maximum(0.0, 1.0 - np.abs(pos[:, None] - j[None, :]))
    w = w / w.sum(axis=1, keepdims=True)
    return w.astype(np.float32)  # [dst, src]


def _build_bass():
    import concourse.bass as bass
    import concourse.tile as tile
    from concourse import bacc, mybir

    f32 = mybir.dt.float32
    f32r = mybir.dt.float32r
    bf16 = mybir.dt.bfloat16
    AF = mybir.ActivationFunctionType
    AX = mybir.AxisListType
    AL = mybir.AluOpType

    nc = bacc.Bacc("TRN2", target_bir_lowering=False, debug=False)

    x_d = nc.dram_tensor("x", [C, H * W], f32, kind="ExternalInput")
    wqk_d = nc.dram_tensor("wqk", [C, 40], bf16, kind="ExternalInput")
    bqk_d = nc.dram_tensor("bqk", [1, 40], bf16, kind="ExternalInput")
    wv_d = nc.dram_tensor("wv", [C, C], bf16, kind="ExternalInput")
    bv_d = nc.dram_tensor("bv", [1, C], bf16, kind="ExternalInput")
    gam_d = nc.dram_tensor("gamma", [1, 1], f32, kind="ExternalInput")
    amat_d = nc.dram_tensor("amat", [128, 8 * H], f32, kind="ExternalInput")
    bmat_d = nc.dram_tensor("bmat", [128, W], f32, kind="ExternalInput")
    y_d = nc.dram_tensor("y", [C, H * W], f32, kind="ExternalOutput")

    with tile.TileContext(nc) as tc:
        with (
            tc.tile_pool(name="xbig", bufs=1) as xbig,
            tc.tile_pool(name="persist", bufs=1) as persist,
        ):
            x0 = xbig.tile([128, H * W], f32)
            x1 = xbig.tile([128, H * W], f32)
            xt = [x0, x1]

            # persistent tensors
            et_sb = persist.tile([128, 8, N], bf16)      # Et[n-chunk][n_l, m]
            vt_sb = persist.tile([128, 8, C + 1], bf16)  # Vt[n-chunk][n_l, c|1]
            o_sb = persist.tile([128, 8, C], bf16)        # O[m-chunk][m_l, c]
            rec_sb = persist.tile([128, 8], f32)
            recg_sb = persist.tile([128, 8], f32)
            a_sb = persist.tile([128, 8 * H], f32)
            b_sb = persist.tile([128, W], f32)
            gam_sb = persist.tile([128, 1], f32)
            wqk_sb = persist.tile([128, 2, 40], bf16)
            bqk_sb = persist.tile([1, 40], bf16)
            wv_sb = persist.tile([128, 2, C], bf16)
            bv_sb = persist.tile([1, C], bf16)
            q_sb = persist.tile([CQ, N], bf16)
            k_sb = persist.tile([CQ, N], bf16)


            arena_pool_cm = tc.tile_pool(name="arena", bufs=3)
            arenas = arena_pool_cm.__enter__()
            arena = {}

            def gen_arena(kc, eng):
                r0 = max(0, 16 * kc - 4)
                r1 = min(128, 16 * kc + 20)
                cnt = r1 - r0
                t_ = arenas.tile([128, 24, W], bf16, tag="arena")
                bb = b_sb[:]
                b_bc = bass.AP(tensor=bb.tensor, offset=bb.offset,
                               ap=[bb.ap[0], [0, cnt], bb.ap[1]])
                aa = a_sb[:, kc * H + r0:kc * H + r1]
                a_bc = bass.AP(tensor=aa.tensor, offset=aa.offset,
                               ap=[aa.ap[0], aa.ap[1], [0, W]])
                eng.tensor_mul(t_[:, 0:cnt, :], b_bc, a_bc)
                arena[kc] = (t_, r0)

            for kc in range(3):
                gen_arena(kc, nc.vector)

            # x input DMAs: strip-interleaved (16 h-rows x both c-halves)
            for st in range(8):
                for t in range(2):
                    sl = bass.ds(st * 2048, 2048)
                    nc.sync.dma_start(out=xt[t][:, sl],
                                      in_=x_d[t * 128:(t + 1) * 128, sl])

            with (
                tc.tile_pool(name="phase1", bufs=1) as ph1,
                tc.tile_pool(name="ptmp", bufs=1) as ptmp,
                tc.tile_pool(name="ps_qk", bufs=1, space="PSUM") as ps_qk,
                tc.tile_pool(name="ps_lt", bufs=4, space="PSUM") as ps_lt,
                tc.tile_pool(name="ps_vt", bufs=2, space="PSUM") as ps_vt,
            ):
                xf_sb = ph1.tile([128, 2, N], bf16)
                ones_sb = ph1.tile([1, N], bf16)
                nc.gpsimd.memset(ones_sb[:], 1.0)

                # ---- pooling: strips of 16 h rows; p1 on gpsimd ----
                for st in range(8):
                    halves = [(0, 2048)] if st != 7 else [(0, 1024),
                                                           (1024, 1024)]
                    for t in range(2):
                        for off, ln in halves:
                            nh = ln // 512
                            strip = xt[t][:, bass.ds(st * 2048 + off, ln)]
                            v1 = strip.rearrange("p (h two w) -> p h two w",
                                                 two=2, w=128)
                            t1 = ptmp.tile([128, nh * 2, 128], bf16,
                                           tag=f"t1_{t}_{ln}")
                            p1eng = nc.gpsimd if t == 0 else nc.vector
                            p1eng.tensor_add(t1[:], v1[:, :, 0, :],
                                             v1[:, :, 1, :])
                            v2 = t1[:].rearrange("p (h two) w -> p h two w",
                                                 two=2)
                            t2 = ptmp.tile([128, nh, 128], bf16,
                                           tag=f"t2_{ln}")
                            nc.vector.tensor_add(t2[:], v2[:, :, 0, :],
                                                 v2[:, :, 1, :])
                            v3 = t2[:].rearrange("p hb (wp two) -> p hb wp two",
                                                 two=2)
                            t3 = ptmp.tile([128, nh, 64], bf16,
                                           tag=f"t3_{ln}")
                            nc.vector.tensor_add(t3[:], v3[:, :, :, 0],
                                                 v3[:, :, :, 1])
                            v4 = t3[:].rearrange("p hb (wb two) -> p hb wb two",
                                                 two=2)
                            xfs = xf_sb[:, t,
                                        bass.ds(st * 128 + off // 16, ln // 16)
                                        ].rearrange("p (hb wb) -> p hb wb",
                                                    hb=nh)
                            nc.vector.tensor_add(xfs, v4[:, :, :, 0],
                                                 v4[:, :, :, 1])

                # ---- q, k (combined M=40, k at partition 32) ----
                qk_ps = ps_qk.tile([40, N], f32)
                for ms in range(2):
                    sl = bass.ds(ms * 512, 512)
                    nc.tensor.matmul(qk_ps[:, sl], wqk_sb[:, 0, :],
                                     xf_sb[:, 0, sl], start=True, stop=False)
                    nc.tensor.matmul(qk_ps[:, sl], wqk_sb[:, 1, :],
                                     xf_sb[:, 1, sl], start=False, stop=False)
                    nc.tensor.matmul(qk_ps[:, sl], bqk_sb[:],
                                     ones_sb[:, sl], start=False, stop=True)
                    nc.scalar.copy(q_sb[:, sl], qk_ps[0:CQ, sl])
                    nc.vector.tensor_copy(k_sb[:, sl], qk_ps[32:40, sl])

                # ---- Vt chunks ----
                nc.gpsimd.memset(vt_sb[:, :, C:C + 1], 1.0)
                for nk in range(8):
                    nsl = bass.ds(nk * 128, 128)
                    vt_ps = ps_vt.tile([128, C], f32, tag="vt")
                    nc.tensor.matmul(vt_ps[:], xf_sb[:, 0, nsl],
                                     wv_sb[:, 0, :], start=True, stop=False)
                    nc.tensor.matmul(vt_ps[:], xf_sb[:, 1, nsl],
                                     wv_sb[:, 1, :], start=False, stop=False)
                    nc.tensor.matmul(vt_ps[:], ones_sb[:, nsl],
                                     bv_sb[:], start=False, stop=True)
                    nc.scalar.copy(vt_sb[:, nk, 0:C], vt_ps[:])

                # ---- logits (transposed) + exp, streamed per 512-half ----
                for ms in range(2):
                    for nk in range(8):
                        nsl = bass.ds(nk * 128, 128)
                        sl = bass.ds(ms * 512, 512)
                        lt_ps = ps_lt.tile([128, 512], f32, tag="lt")
                        nc.tensor.matmul(lt_ps[:], k_sb[:, nsl],
                                         q_sb[:, sl], start=True, stop=True)
                        nc.scalar.activation(et_sb[:, nk, sl], lt_ps[:],
                                             func=AF.Exp)

            # ---- attn @ V ----
            with tc.tile_pool(name="ps_o", bufs=3, space="PSUM") as ps_o:
                for mk in range(8):
                    msl = bass.ds(mk * 128, 128)
                    o_ps = ps_o.tile([128, C + 1], f32, tag="o")
                    for nk in range(8):
                        nc.tensor.matmul(o_ps[:], et_sb[:, nk, msl], vt_sb[:, nk, :],
                                         start=(nk == 0), stop=(nk == 7))
                    nc.vector.reciprocal(rec_sb[:, mk:mk + 1], o_ps[:, C:C + 1])
                    nc.vector.tensor_scalar_mul(recg_sb[:, mk:mk + 1],
                                                rec_sb[:, mk:mk + 1],
                                                gam_sb[:, 0:1])
                    nc.scalar.mul(o_sb[:, mk, :], o_ps[:, 0:C],
                                  mul=recg_sb[:, mk:mk + 1])

            # ---- fused upsample + residual + output ----
            # arena_k[p, j, w] = A[p, kH + r0 + j] * B[p, w]; the moving
            # operand for slice s, chunk k is arena_k[:, 4s-r0 : 4s-r0+4, :].
            with (
                tc.tile_pool(name="ps_y", bufs=4, space="PSUM") as ps_y,
            ):
                def slice_chunks(s):
                    hbs = {hb for hb in (s - 1, s, s + 1) if 0 <= hb < 32}
                    return sorted({hb // 4 for hb in hbs})

                for g in range(16):  # 2 slices per psum tile
                    for kc in slice_chunks(2 * g + 1):
                        if kc not in arena:
                            gen_arena(kc, nc.gpsimd)
                    for ch in range(2):
                        y_ps = ps_y.tile([128, 1024], f32, tag="y")
                        for q in range(2):
                            s = 2 * g + q
                            ks = slice_chunks(s)
                            for i, kc in enumerate(ks):
                                t_, r0 = arena[kc]
                                nc.tensor.matmul(
                                    y_ps[:, bass.ds(q * 512, 512)],
                                    o_sb[:, kc, ch * 128:(ch + 1) * 128],
                                    t_[:, 4 * s - r0:4 * s - r0 + 4, :],
                                    start=(i == 0), stop=(i == len(ks) - 1))
                        osl = bass.ds(g * 1024, 1024)
                        nc.vector.tensor_add(xt[ch][:, osl], y_ps[:], xt[ch][:, osl])
                    if g % 2 == 1:
                        gg = g // 2
                        gsl = bass.ds(gg * 2048, 2048)
                        for ch in range(2):
                            nc.sync.dma_start(out=y_d[ch * 128:(ch + 1) * 128, gsl],
                                              in_=xt[ch][:, gsl])
            arena_pool_cm.__exit__(None, None, None)
    nc.compile()
    return nc


def _get_nc():
    if "nc" not in _CACHE:
        _CACHE["nc"] = _build_bass()
    return _CACHE["nc"]


def kernel(x, Wq, bq, Wk, bk, Wv, bv, gamma):
    from concourse.bass_utils import run_bass_kernel_spmd

    x = np.ascontiguousarray(np.asarray(x, dtype=np.float32))
    U = _resize_matrix(H, HD)  # [128, 32]

    p = np.arange(128)
    amat = np.zeros((128, 8 * H), dtype=np.float32)
    for kc in range(8):
        amat[:, kc * H:(kc + 1) * H] = U[:, 4 * kc + p // 32].T
    bmat = np.ascontiguousarray(U[:, p % 32].T)  # [128, W] -> B[p, w] = U[w, p%32]

    import ml_dtypes
    bfd = ml_dtypes.bfloat16
    wqk = np.zeros((C, 40), dtype=bfd)
    wqk[:, 0:8] = (np.asarray(Wq).T / 16.0).astype(bfd)
    wqk[:, 32:40] = (np.asarray(Wk).T / 16.0).astype(bfd)
    bqk = np.zeros((1, 40), dtype=bfd)
    bqk[0, 0:8] = np.asarray(bq).astype(bfd)
    bqk[0, 32:40] = np.asarray(bk).astype(bfd)
    wv = np.ascontiguousarray(np.asarray(Wv).T / 16.0).astype(bfd)
    bvr = np.asarray(bv)[None, :].astype(bfd)
    gam = np.asarray(gamma).reshape(1, 1).astype(np.float32)

    nc = _get_nc()
    in_maps = []
    for i in range(NCORES):
        in_maps.append({
            "x": np.ascontiguousarray(x[i].reshape(C, H * W)),
            "wqk": wqk, "bqk": bqk, "wv": wv, "bv": bvr,
            "gamma": gam, "amat": amat, "bmat": bmat,
        })
    res = run_bass_kernel_spmd(nc, in_maps, core_ids=list(range(NCORES)))
    y = np.stack([r["y"].reshape(C, H, W) for r in res.results])
    return y.astype(np.float32)


if __name__ == "__main__":
    rng = np.random.default_rng(0)
    inputs = {
        "x": rng.standard_normal((B, C, H, W), dtype=np.float32),
        "Wq": rng.standard_normal((CQ, C), dtype=np.float32) * 0.05,
        "bq": rng.standard_normal((CQ,), dtype=np.float32) * 0.05,
        "Wk": rng.standard_normal((CQ, C), dtype=np.float32) * 0.05,
        "bk": rng.standard_normal((CQ,), dtype=np.float32) * 0.05,
        "Wv": rng.standard_normal((C, C), dtype=np.float32) * 0.05,
        "bv": rng.standard_normal((C,), dtype=np.float32) * 0.05,
        "gamma": np.zeros((1,), dtype=np.float32),
    }
    y = kernel(**inputs)
    print("out", y.shape, y.dtype, float(np.abs(y - inputs["x"]).max()))


# revision 18
# speedup vs baseline: 339.1561x; 1.0110x over previous
"""Trainium2 Bass kernel for the AttentionBlock problem.

Math (per batch b):
  x_down = avgpool4x4(x)            # [C, 32, 32] -> xf [C, N], N=1024
  q,k = Wq/Wk @ xf + b              # [8, N]
  v = Wv @ xf + bv                  # [C, N]
  attn = softmax_n(q^T k)           # [N, N]
  out[c,m] = sum_n v[c,n] attn[m,n]
  y = gamma * upsample_bilinear(out) + x

Mapping (one NeuronCore per batch, 8 cores):
  - x resident in SBUF as two [128, 16384] tiles (c-halves); pooled via DVE
    (w-reduce + h pair adds). The 1/16 mean factor is folded into the weights.
  - q,k computed together (lhsT = [WqT|WkT], M=16), f32r matmuls.
  - logits computed transposed: Lt[n, m] = k^T q, n on partitions -> exp on ACT
    -> Et bf16. Vt[n, c] computed directly (lhsT = xf chunk), ones column
    appended for the softmax denominator.
  - attn@V: O[m, c] = Et^T Vt accumulated over n-chunks; denominator lands in
    column 256; normalize via per-partition reciprocal * tensor_scalar.
  - Upsample fused into one sparse matmul: y[c, (H,W)-slice] = sum_m O[m, c] *
    slab[m, slice], slab[m, (H,W)] = gamma*U[H, hb(m)]*U[W, wb(m)] generated
    on-device (4 ACT ops per slab). Residual add fused into the PSUM->SBUF
    copy (DVE tensor_add with x), written back in place over x, then DMA out.
"""

import numpy as np

B, C, H, W = 8, 256, 128, 128
HD, WD = 32, 32
N = HD * WD  # 1024
CQ = 8
NCORES = 8

_CACHE = {}


def _resize_matrix(dst: int, src: int) -> np.ndarray:
    """Bilinear (half-pixel, edge-renormalized) resize matrix, matches
    jax.image.resize(method='linear') for upsampling."""
    scale = dst / src
    pos = (np.arange(dst, dtype=np.float64) + 0.5) / scale - 0.5
    j = np.arange(src, dtype=np.float64)
    w = np.maximum(0.0, 1.0 - np.abs(pos[:, None] - j[None, :]))
    w = w / w.sum(axis=1, keepdims=True)
    return w.astype(np.float32)  # [dst, src]


def _build_bass():
    import concourse.bass as bass
    import concourse.tile as tile
    from concourse import bacc, mybir

    f32 = mybir.dt.float32
    f32r = mybir.dt.float32r
    bf16 = mybir.dt.bfloat16
    AF = mybir.ActivationFunctionType
    AX = mybir.AxisListType
    AL = mybir.AluOpType

    nc = bacc.Bacc("TRN2", target_bir_lowering=False, debug=False)

    x_d = nc.dram_tensor("x", [C, H * W], f32, kind="ExternalInput")
    wqk_d = nc.dram_tensor("wqk", [C, 40], bf16, kind="ExternalInput")
    bqk_d = nc.dram_tensor("bqk", [1, 40], bf16, kind="ExternalInput")
    wv_d = nc.dram_tensor("wv", [C, C], bf16, kind="ExternalInput")
    bv_d = nc.dram_tensor("bv", [1, C], bf16, kind="ExternalInput")
    gam_d = nc.dram_tensor("gamma", [1, 1], f32, kind="ExternalInput")
    amat_d = nc.dram_tensor("amat", [128, 8 * H], f32, kind="ExternalInput")
    bmat_d = nc.dram_tensor("bmat", [128, W], f32, kind="ExternalInput")
    y_d = nc.dram_tensor("y", [C, H * W], f32, kind="ExternalOutput")

    with tile.TileContext(nc) as tc:
        with (
            tc.tile_pool(name="xbig", bufs=1) as xbig,
            tc.tile_pool(name="persist", bufs=1) as persist,
        ):
            x0 = xbig.tile([128, H * W], f32)
            x1 = xbig.tile([128, H * W], f32)
            xt = [x0, x1]

            # persistent tensors
            et_sb = persist.tile([128, 8, N], bf16)      # Et[n-chunk][n_l, m]
            vt_sb = persist.tile([128, 8, C + 1], bf16)  # Vt[n-chunk][n_l, c|1]
            o_sb = persist.tile([128, 8, C], bf16)        # O[m-chunk][m_l, c]
            rec_sb = persist.tile([128, 8], f32)
            recg_sb = persist.tile([128, 8], f32)
            a_sb = persist.tile([128, 8 * H], f32)
            b_sb = persist.tile([128, W], f32)
            gam_sb = persist.tile([128, 1], f32)
            wqk_sb = persist.tile([128, 2, 40], bf16)
            bqk_sb = persist.tile([1, 40], bf16)
            wv_sb = persist.tile([128, 2, C], bf16)
            bv_sb = persist.tile([1, C], bf16)
            q_sb = persist.tile([CQ, N], bf16)
            k_sb = persist.tile([CQ, N], bf16)


            arena_pool_cm = tc.tile_pool(name="arena", bufs=3)
            arenas = arena_pool_cm.__enter__()
            arena = {}

            def gen_arena(kc, eng):
                r0 = max(0, 16 * kc - 4)
                r1 = min(128, 16 * kc + 20)
                cnt = r1 - r0
                t_ = arenas.tile([128, 24, W], bf16, tag="arena")
                bb = b_sb[:]
                b_bc = bass.AP(tensor=bb.tensor, offset=bb.offset,
                               ap=[bb.ap[0], [0, cnt], bb.ap[1]])
                aa = a_sb[:, kc * H + r0:kc * H + r1]
                a_bc = bass.AP(tensor=aa.tensor, offset=aa.offset,
                               ap=[aa.ap[0], aa.ap[1], [0, W]])
                eng.tensor_mul(t_[:, 0:cnt, :], b_bc, a_bc)
                arena[kc] = (t_, r0)

            for kc in range(3):
                gen_arena(kc, nc.vector)

            # x input DMAs: strip-interleaved (16 h-rows x both c-halves)
            for st in range(8):
                for t in range(2):
                    sl = bass.ds(st * 2048, 2048)
                    nc.sync.dma_start(out=xt[t][:, sl],
                                      in_=x_d[t * 128:(t + 1) * 128, sl])

            with (
                tc.tile_pool(name="phase1", bufs=1) as ph1,
                tc.tile_pool(name="ptmp", bufs=1) as ptmp,
                tc.tile_pool(name="ps_qk", bufs=1, space="PSUM") as ps_qk,
                tc.tile_pool(name="ps_lt", bufs=4, space="PSUM") as ps_lt,
                tc.tile_pool(name="ps_vt", bufs=2, space="PSUM") as ps_vt,
            ):
                xf_sb = ph1.tile([128, 2, N], bf16)
                ones_sb = ph1.tile([1, N], bf16)
                nc.gpsimd.memset(ones_sb[:], 1.0)

                # ---- pooling: strips of 16 h rows; p1 on gpsimd ----
                for st in range(8):
                    halves = [(0, 2048)] if st != 7 else [(0, 1024),
                                                           (1024, 1024)]
                    for t in range(2):
                        for off, ln in halves:
                            nh = ln // 512
                            strip = xt[t][:, bass.ds(st * 2048 + off, ln)]
                            v1 = strip.rearrange("p (h two w) -> p h two w",
                                                 two=2, w=128)
                            t1 = ptmp.tile([128, nh * 2, 128], bf16,
                                           tag=f"t1_{t}_{ln}")
                            p1eng = nc.gpsimd if t == 0 else nc.vector
                            rest = p1eng if st == 7 else nc.vector
                            p1eng.tensor_add(t1[:], v1[:, :, 0, :],
                                             v1[:, :, 1, :])
                            v2 = t1[:].rearrange("p (h two) w -> p h two w",
                                                 two=2)
                            t2 = ptmp.tile([128, nh, 128], bf16,
                                           tag=f"t2_{t}_{ln}")
                            rest.tensor_add(t2[:], v2[:, :, 0, :],
                                            v2[:, :, 1, :])
                            v3 = t2[:].rearrange("p hb (wp two) -> p hb wp two",
                                                 two=2)
                            t3 = ptmp.tile([128, nh, 64], bf16,
                                           tag=f"t3_{t}_{ln}")
                            rest.tensor_add(t3[:], v3[:, :, :, 0],
                                            v3[:, :, :, 1])
                            v4 = t3[:].rearrange("p hb (wb two) -> p hb wb two",
                                                 two=2)
                            xfs = xf_sb[:, t,
                                        bass.ds(st * 128 + off // 16, ln // 16)
                                        ].rearrange("p (hb wb) -> p hb wb",
                                                    hb=nh)
                            rest.tensor_add(xfs, v4[:, :, :, 0],
                                            v4[:, :, :, 1])

                # ---- q, k (combined M=40, k at partition 32) ----
                qk_ps = ps_qk.tile([40, N], f32)
                for ms in range(2):
                    sl = bass.ds(ms * 512, 512)
                    nc.tensor.matmul(qk_ps[:, sl], wqk_sb[:, 0, :],
                                     xf_sb[:, 0, sl], start=True, stop=False)
                    nc.tensor.matmul(qk_ps[:, sl], wqk_sb[:, 1, :],
                                     xf_sb[:, 1, sl], start=False, stop=False)
                    nc.tensor.matmul(qk_ps[:, sl], bqk_sb[:],
                                     ones_sb[:, sl], start=False, stop=True)
                    nc.scalar.copy(q_sb[:, sl], qk_ps[0:CQ, sl])
                    nc.vector.tensor_copy(k_sb[:, sl], qk_ps[32:40, sl])

                # ---- Vt chunks ----
                nc.gpsimd.memset(vt_sb[:, :, C:C + 1], 1.0)
                for nk in range(8):
                    nsl = bass.ds(nk * 128, 128)
                    vt_ps = ps_vt.tile([128, C], f32, tag="vt")
                    nc.tensor.matmul(vt_ps[:], xf_sb[:, 0, nsl],
                                     wv_sb[:, 0, :], start=True, stop=False)
                    nc.tensor.matmul(vt_ps[:], xf_sb[:, 1, nsl],
                                     wv_sb[:, 1, :], start=False, stop=False)
                    nc.tensor.matmul(vt_ps[:], ones_sb[:, nsl],
                                     bv_sb[:], start=False, stop=True)
                    nc.scalar.copy(vt_sb[:, nk, 0:C], vt_ps[:])

                # ---- logits (transposed) + exp, streamed per 512-half ----
                for ms in range(2):
                    for nk in range(8):
                        nsl = bass.ds(nk * 128, 128)
                        sl = bass.ds(ms * 512, 512)
                        lt_ps = ps_lt.tile([128, 512], f32, tag="lt")
                        nc.tensor.matmul(lt_ps[:], k_sb[:, nsl],
                                         q_sb[:, sl], start=True, stop=True)
                        nc.scalar.activation(et_sb[:, nk, sl], lt_ps[:],
                                             func=AF.Exp)

            # ---- attn @ V ----
            with tc.tile_pool(name="ps_o", bufs=3, space="PSUM") as ps_o:
                for mk in range(8):
                    msl = bass.ds(mk * 128, 128)
                    o_ps = ps_o.tile([128, C + 1], f32, tag="o")
                    for nk in range(8):
                        nc.tensor.matmul(o_ps[:], et_sb[:, nk, msl], vt_sb[:, nk, :],
                                         start=(nk == 0), stop=(nk == 7))
                    nc.vector.reciprocal(rec_sb[:, mk:mk + 1], o_ps[:, C:C + 1])
                    if mk == 0:
                        nc.vector.tensor_scalar(
                            out=o_sb[:, mk, :], in0=o_ps[:, 0:C],
                            scalar1=rec_sb[:, mk:mk + 1],
                            scalar2=gam_sb[:, 0:1],
                            op0=AL.mult, op1=AL.mult)
                    else:
                        nc.vector.tensor_scalar_mul(recg_sb[:, mk:mk + 1],
                                                    rec_sb[:, mk:mk + 1],
                                                    gam_sb[:, 0:1])
                        nc.scalar.mul(o_sb[:, mk, :], o_ps[:, 0:C],
                                      mul=recg_sb[:, mk:mk + 1])

            # ---- fused upsample + residual + output ----
            # arena_k[p, j, w] = A[p, kH + r0 + j] * B[p, w]; the moving
            # operand for slice s, chunk k is arena_k[:, 4s-r0 : 4s-r0+4, :].
            with (
                tc.tile_pool(name="ps_y", bufs=4, space="PSUM") as ps_y,
            ):
                def slice_chunks(s):
                    hbs = {hb for hb in (s - 1, s, s + 1) if 0 <= hb < 32}
                    return sorted({hb // 4 for hb in hbs})

                for g in range(16):  # 2 slices per psum tile
                    for kc in slice_chunks(2 * g + 1):
                        if kc not in arena:
                            gen_arena(kc, nc.gpsimd)
                    for ch in range(2):
                        y_ps = ps_y.tile([128, 1024], f32, tag="y")
                        for q in range(2):
                            s = 2 * g + q
                            ks = slice_chunks(s)
                            for i, kc in enumerate(ks):
                                t_, r0 = arena[kc]
                                nc.tensor.matmul(
                                    y_ps[:, bass.ds(q * 512, 512)],
                                    o_sb[:, kc, ch * 128:(ch + 1) * 128],
                                    t_[:, 4 * s - r0:4 * s - r0 + 4, :],
                                    start=(i == 0), stop=(i == len(ks) - 1))
                        osl = bass.ds(g * 1024, 1024)
                        nc.vector.tensor_add(xt[ch][:, osl], y_ps[:], xt[ch][:, osl])
                    if g % 2 == 1:
                        gg = g // 2
                        gsl = bass.ds(gg * 2048, 2048)
                        for ch in range(2):
                            nc.sync.dma_start(out=y_d[ch * 128:(ch + 1) * 128, gsl],
                                              in_=xt[ch][:, gsl])
            arena_pool_cm.__exit__(None, None, None)
    nc.compile()
    return nc


def _get_nc():
    if "nc" not in _CACHE:
        _CACHE["nc"] = _build_bass()
    return _CACHE["nc"]


def kernel(x, Wq, bq, Wk, bk, Wv, bv, gamma):
    from concourse.bass_utils import run_bass_kernel_spmd

    x = np.ascontiguousarray(np.asarray(x, dtype=np.float32))
    U = _resize_matrix(H, HD)  # [128, 32]

    p = np.arange(128)
    amat = np.zeros((128, 8 * H), dtype=np.float32)
    for kc in range(8):
        amat[:, kc * H:(kc + 1) * H] = U[:, 4 * kc + p // 32].T
    bmat = np.ascontiguousarray(U[:, p % 32].T)  # [128, W] -> B[p, w] = U[w, p%32]

    import ml_dtypes
    bfd = ml_dtypes.bfloat16
    wqk = np.zeros((C, 40), dtype=bfd)
    wqk[:, 0:8] = (np.asarray(Wq).T / 16.0).astype(bfd)
    wqk[:, 32:40] = (np.asarray(Wk).T / 16.0).astype(bfd)
    bqk = np.zeros((1, 40), dtype=bfd)
    bqk[0, 0:8] = np.asarray(bq).astype(bfd)
    bqk[0, 32:40] = np.asarray(bk).astype(bfd)
    wv = np.ascontiguousarray(np.asarray(Wv).T / 16.0).astype(bfd)
    bvr = np.asarray(bv)[None, :].astype(bfd)
    gam = np.asarray(gamma).reshape(1, 1).astype(np.float32)

    nc = _get_nc()
    in_maps = []
    for i in range(NCORES):
        in_maps.append({
            "x": np.ascontiguousarray(x[i].reshape(C, H * W)),
            "wqk": wqk, "bqk": bqk, "wv": wv, "bv": bvr,
            "gamma": gam, "amat": amat, "bmat": bmat,
        })
    res = run_bass_kernel_spmd(nc, in_maps, core_ids=list(range(NCORES)))
    y = np.stack([r["y"].reshape(C, H, W) for r in res.results])
    return y.astype(np.float32)


if __name__ == "__main__":
    rng = np.random.default_rng(0)
    inputs = {
        "x": rng.standard_normal((B, C, H, W), dtype=np.float32),
        "Wq": rng.standard_normal((CQ, C), dtype=np.float32) * 0.05,
        "bq": rng.standard_normal((CQ,), dtype=np.float32) * 0.05,
        "Wk": rng.standard_normal((CQ, C), dtype=np.float32) * 0.05,
        "bk": rng.standard_normal((CQ,), dtype=np.float32) * 0.05,
        "Wv": rng.standard_normal((C, C), dtype=np.float32) * 0.05,
        "bv": rng.standard_normal((C,), dtype=np.float32) * 0.05,
        "gamma": np.zeros((1,), dtype=np.float32),
    }
    y = kernel(**inputs)
    print("out", y.shape, y.dtype, float(np.abs(y - inputs["x"]).max()))


# revision 19
# speedup vs baseline: 342.5621x; 1.0100x over previous
"""Trainium2 Bass kernel for the AttentionBlock problem.

Math (per batch b):
  x_down = avgpool4x4(x)            # [C, 32, 32] -> xf [C, N], N=1024
  q,k = Wq/Wk @ xf + b              # [8, N]
  v = Wv @ xf + bv                  # [C, N]
  attn = softmax_n(q^T k)           # [N, N]
  out[c,m] = sum_n v[c,n] attn[m,n]
  y = gamma * upsample_bilinear(out) + x

Mapping (one NeuronCore per batch, 8 cores):
  - x resident in SBUF as two [128, 16384] tiles (c-halves); pooled via DVE
    (w-reduce + h pair adds). The 1/16 mean factor is folded into the weights.
  - q,k computed together (lhsT = [WqT|WkT], M=16), f32r matmuls.
  - logits computed transposed: Lt[n, m] = k^T q, n on partitions -> exp on ACT
    -> Et bf16. Vt[n, c] computed directly (lhsT = xf chunk), ones column
    appended for the softmax denominator.
  - attn@V: O[m, c] = Et^T Vt accumulated over n-chunks; denominator lands in
    column 256; normalize via per-partition reciprocal * tensor_scalar.
  - Upsample fused into one sparse matmul: y[c, (H,W)-slice] = sum_m O[m, c] *
    slab[m, slice], slab[m, (H,W)] = gamma*U[H, hb(m)]*U[W, wb(m)] generated
    on-device (4 ACT ops per slab). Residual add fused into the PSUM->SBUF
    copy (DVE tensor_add with x), written back in place over x, then DMA out.
"""

import numpy as np

B, C, H, W = 8, 256, 128, 128
HD, WD = 32, 32
N = HD * WD  # 1024
CQ = 8
NCORES = 8

_CACHE = {}


def _resize_matrix(dst: int, src: int) -> np.ndarray:
    """Bilinear (half-pixel, edge-renormalized) resize matrix, matches
    jax.image.resize(method='linear') for upsampling."""
    scale = dst / src
    pos = (np.arange(dst, dtype=np.float64) + 0.5) / scale - 0.5
    j = np.arange(src, dtype=np.float64)
    w = np.maximum(0.0, 1.0 - np.abs(pos[:, None] - j[None, :]))
    w = w / w.sum(axis=1, keepdims=True)
    return w.astype(np.float32)  # [dst, src]


def _build_bass():
    import concourse.bass as bass
    import concourse.tile as tile
    from concourse import bacc, mybir

    f32 = mybir.dt.float32
    f32r = mybir.dt.float32r
    bf16 = mybir.dt.bfloat16
    AF = mybir.ActivationFunctionType
    AX = mybir.AxisListType
    AL = mybir.AluOpType

    nc = bacc.Bacc("TRN2", target_bir_lowering=False, debug=False)

    x_d = nc.dram_tensor("x", [C, H * W], f32, kind="ExternalInput")
    wqk_d = nc.dram_tensor("wqk", [C, 40], bf16, kind="ExternalInput")
    bqk_d = nc.dram_tensor("bqk", [1, 40], bf16, kind="ExternalInput")
    wv_d = nc.dram_tensor("wv", [C, C], bf16, kind="ExternalInput")
    bv_d = nc.dram_tensor("bv", [1, C], bf16, kind="ExternalInput")
    gam_d = nc.dram_tensor("gamma", [1, 1], f32, kind="ExternalInput")
    amat_d = nc.dram_tensor("amat", [128, 8 * H], bf16, kind="ExternalInput")
    bmat_d = nc.dram_tensor("bmat", [128, W], bf16, kind="ExternalInput")
    y_d = nc.dram_tensor("y", [C, H * W], f32, kind="ExternalOutput")

    with tile.TileContext(nc) as tc:
        with (
            tc.tile_pool(name="xbig", bufs=1) as xbig,
            tc.tile_pool(name="persist", bufs=1) as persist,
        ):
            x0 = xbig.tile([128, H * W], f32)
            x1 = xbig.tile([128, H * W], f32)
            xt = [x0, x1]

            # persistent tensors
            et_sb = persist.tile([128, 8, N], bf16)      # Et[n-chunk][n_l, m]
            vt_sb = persist.tile([128, 8, C + 1], bf16)  # Vt[n-chunk][n_l, c|1]
            o_sb = persist.tile([128, 8, C], bf16)        # O[m-chunk][m_l, c]
            rec_sb = persist.tile([128, 8], f32)
            recg_sb = persist.tile([128, 8], f32)
            a_sb = persist.tile([128, 8 * H], bf16)
            b_sb = persist.tile([128, W], bf16)
            gam_sb = persist.tile([128, 1], f32)
            wqk_sb = persist.tile([128, 2, 40], bf16)
            bqk_sb = persist.tile([1, 40], bf16)
            wv_sb = persist.tile([128, 2, C], bf16)
            bv_sb = persist.tile([1, C], bf16)
            q_sb = persist.tile([CQ, N], bf16)
            k_sb = persist.tile([CQ, N], bf16)


            arena_pool_cm = tc.tile_pool(name="arena", bufs=3)
            arenas = arena_pool_cm.__enter__()
            arena = {}

            def gen_arena(kc, eng):
                r0 = max(0, 16 * kc - 4)
                r1 = min(128, 16 * kc + 20)
                cnt = r1 - r0
                t_ = arenas.tile([128, 24, W], bf16, tag="arena")
                bb = b_sb[:]
                b_bc = bass.AP(tensor=bb.tensor, offset=bb.offset,
                               ap=[bb.ap[0], [0, cnt], bb.ap[1]])
                aa = a_sb[:, kc * H + r0:kc * H + r1]
                a_bc = bass.AP(tensor=aa.tensor, offset=aa.offset,
                               ap=[aa.ap[0], aa.ap[1], [0, W]])
                eng.tensor_mul(t_[:, 0:cnt, :], b_bc, a_bc)
                arena[kc] = (t_, r0)

            for kc in range(3):
                gen_arena(kc, nc.vector)

            # x input DMAs: strip-interleaved (16 h-rows x both c-halves)
            for st in range(8):
                for t in range(2):
                    sl = bass.ds(st * 2048, 2048)
                    nc.sync.dma_start(out=xt[t][:, sl],
                                      in_=x_d[t * 128:(t + 1) * 128, sl])

            with (
                tc.tile_pool(name="phase1", bufs=1) as ph1,
                tc.tile_pool(name="ptmp", bufs=1) as ptmp,
                tc.tile_pool(name="ps_qk", bufs=1, space="PSUM") as ps_qk,
                tc.tile_pool(name="ps_lt", bufs=4, space="PSUM") as ps_lt,
                tc.tile_pool(name="ps_vt", bufs=2, space="PSUM") as ps_vt,
            ):
                xf_sb = ph1.tile([128, 2, N], bf16)
                ones_sb = ph1.tile([1, N], bf16)
                nc.gpsimd.memset(ones_sb[:], 1.0)

                # ---- pooling: strips of 16 h rows; p1 on gpsimd ----
                for st in range(8):
                    halves = [(0, 2048)] if st != 7 else [(0, 1024),
                                                           (1024, 1024)]
                    for t in range(2):
                        for off, ln in halves:
                            nh = ln // 512
                            strip = xt[t][:, bass.ds(st * 2048 + off, ln)]
                            v1 = strip.rearrange("p (h two w) -> p h two w",
                                                 two=2, w=128)
                            t1 = ptmp.tile([128, nh * 2, 128], bf16,
                                           tag=f"t1_{t}_{ln}")
                            p1eng = nc.gpsimd if t == 0 else nc.vector
                            rest = p1eng if st == 7 else nc.vector
                            p1eng.tensor_add(t1[:], v1[:, :, 0, :],
                                             v1[:, :, 1, :])
                            v2 = t1[:].rearrange("p (h two) w -> p h two w",
                                                 two=2)
                            t2 = ptmp.tile([128, nh, 128], bf16,
                                           tag=f"t2_{t}_{ln}")
                            rest.tensor_add(t2[:], v2[:, :, 0, :],
                                            v2[:, :, 1, :])
                            v3 = t2[:].rearrange("p hb (wp two) -> p hb wp two",
                                                 two=2)
                            t3 = ptmp.tile([128, nh, 64], bf16,
                                           tag=f"t3_{t}_{ln}")
                            rest.tensor_add(t3[:], v3[:, :, :, 0],
                                            v3[:, :, :, 1])
                            v4 = t3[:].rearrange("p hb (wb two) -> p hb wb two",
                                                 two=2)
                            xfs = xf_sb[:, t,
                                        bass.ds(st * 128 + off // 16, ln // 16)
                                        ].rearrange("p (hb wb) -> p hb wb",
                                                    hb=nh)
                            rest.tensor_add(xfs, v4[:, :, :, 0],
                                            v4[:, :, :, 1])

                # ---- q, k (combined M=40, k at partition 32) ----
                qk_ps = ps_qk.tile([40, N], f32)
                for ms in range(2):
                    sl = bass.ds(ms * 512, 512)
                    nc.tensor.matmul(qk_ps[:, sl], wqk_sb[:, 0, :],
                                     xf_sb[:, 0, sl], start=True, stop=False)
                    nc.tensor.matmul(qk_ps[:, sl], wqk_sb[:, 1, :],
                                     xf_sb[:, 1, sl], start=False, stop=False)
                    nc.tensor.matmul(qk_ps[:, sl], bqk_sb[:],
                                     ones_sb[:, sl], start=False, stop=True)
                    nc.scalar.copy(q_sb[:, sl], qk_ps[0:CQ, sl])
                    nc.vector.tensor_copy(k_sb[:, sl], qk_ps[32:40, sl])

                # ---- Vt chunks ----
                nc.gpsimd.memset(vt_sb[:, :, C:C + 1], 1.0)
                for nk in range(8):
                    nsl = bass.ds(nk * 128, 128)
                    vt_ps = ps_vt.tile([128, C], f32, tag="vt")
                    nc.tensor.matmul(vt_ps[:], xf_sb[:, 0, nsl],
                                     wv_sb[:, 0, :], start=True, stop=False)
                    nc.tensor.matmul(vt_ps[:], xf_sb[:, 1, nsl],
                                     wv_sb[:, 1, :], start=False, stop=False)
                    nc.tensor.matmul(vt_ps[:], ones_sb[:, nsl],
                                     bv_sb[:], start=False, stop=True)
                    nc.scalar.copy(vt_sb[:, nk, 0:C], vt_ps[:])

                # ---- logits (transposed) + exp, streamed per 512-half ----
                for ms in range(2):
                    for nk in range(8):
                        nsl = bass.ds(nk * 128, 128)
                        sl = bass.ds(ms * 512, 512)
                        lt_ps = ps_lt.tile([128, 512], f32, tag="lt")
                        nc.tensor.matmul(lt_ps[:], k_sb[:, nsl],
                                         q_sb[:, sl], start=True, stop=True)
                        nc.scalar.activation(et_sb[:, nk, sl], lt_ps[:],
                                             func=AF.Exp)

            # ---- attn @ V ----
            with tc.tile_pool(name="ps_o", bufs=3, space="PSUM") as ps_o:
                for mk in range(8):
                    msl = bass.ds(mk * 128, 128)
                    o_ps = ps_o.tile([128, C + 1], f32, tag="o")
                    for nk in range(8):
                        nc.tensor.matmul(o_ps[:], et_sb[:, nk, msl], vt_sb[:, nk, :],
                                         start=(nk == 0), stop=(nk == 7))
                    nc.vector.reciprocal(rec_sb[:, mk:mk + 1], o_ps[:, C:C + 1])
                    if mk == 0:
                        nc.vector.tensor_scalar(
                            out=o_sb[:, mk, :], in0=o_ps[:, 0:C],
                            scalar1=rec_sb[:, mk:mk + 1],
                            scalar2=gam_sb[:, 0:1],
                            op0=AL.mult, op1=AL.mult)
                    else:
                        nc.vector.tensor_scalar_mul(recg_sb[:, mk:mk + 1],
                                                    rec_sb[:, mk:mk + 1],
                                                    gam_sb[:, 0:1])
                        nc.scalar.mul(o_sb[:, mk, :], o_ps[:, 0:C],
                                      mul=recg_sb[:, mk:mk + 1])

            # ---- fused upsample + residual + output ----
            # arena_k[p, j, w] = A[p, kH + r0 + j] * B[p, w]; the moving
            # operand for slice s, chunk k is arena_k[:, 4s-r0 : 4s-r0+4, :].
            with (
                tc.tile_pool(name="ps_y", bufs=4, space="PSUM") as ps_y,
            ):
                def slice_chunks(s):
                    hbs = {hb for hb in (s - 1, s, s + 1) if 0 <= hb < 32}
                    return sorted({hb // 4 for hb in hbs})

                for g in range(16):  # 2 slices per psum tile
                    for kc in slice_chunks(2 * g + 1):
                        if kc not in arena:
                            gen_arena(kc, nc.gpsimd)
                    for ch in range(2):
                        y_ps = ps_y.tile([128, 1024], f32, tag="y")
                        for q in range(2):
                            s = 2 * g + q
                            ks = slice_chunks(s)
                            for i, kc in enumerate(ks):
                                t_, r0 = arena[kc]
                                nc.tensor.matmul(
                                    y_ps[:, bass.ds(q * 512, 512)],
                                    o_sb[:, kc, ch * 128:(ch + 1) * 128],
                                    t_[:, 4 * s - r0:4 * s - r0 + 4, :],
                                    start=(i == 0), stop=(i == len(ks) - 1))
                        osl = bass.ds(g * 1024, 1024)
                        nc.vector.tensor_add(xt[ch][:, osl], y_ps[:], xt[ch][:, osl])
                    if g % 2 == 1:
                        gg = g // 2
                        gsl = bass.ds(gg * 2048, 2048)
                        for ch in range(2):
                            nc.sync.dma_start(out=y_d[ch * 128:(ch + 1) * 128, gsl],
                                              in_=xt[ch][:, gsl])
            arena_pool_cm.__exit__(None, None, None)
    nc.compile()
    return nc


def _get_nc():
    if "nc" not in _CACHE:
        _CACHE["nc"] = _build_bass()
    return _CACHE["nc"]


def kernel(x, Wq, bq, Wk, bk, Wv, bv, gamma):
    from concourse.bass_utils import run_bass_kernel_spmd

    x = np.ascontiguousarray(np.asarray(x, dtype=np.float32))
    U = _resize_matrix(H, HD)  # [128, 32]

    import ml_dtypes as _mld
    p = np.arange(128)
    amat = np.zeros((128, 8 * H), dtype=_mld.bfloat16)
    for kc in range(8):
        amat[:, kc * H:(kc + 1) * H] = U[:, 4 * kc + p // 32].T.astype(_mld.bfloat16)
    bmat = np.ascontiguousarray(U[:, p % 32].T.astype(_mld.bfloat16))

    import ml_dtypes
    bfd = ml_dtypes.bfloat16
    wqk = np.zeros((C, 40), dtype=bfd)
    wqk[:, 0:8] = (np.asarray(Wq).T / 16.0).astype(bfd)
    wqk[:, 32:40] = (np.asarray(Wk).T / 16.0).astype(bfd)
    bqk = np.zeros((1, 40), dtype=bfd)
    bqk[0, 0:8] = np.asarray(bq).astype(bfd)
    bqk[0, 32:40] = np.asarray(bk).astype(bfd)
    wv = np.ascontiguousarray(np.asarray(Wv).T / 16.0).astype(bfd)
    bvr = np.asarray(bv)[None, :].astype(bfd)
    gam = np.asarray(gamma).reshape(1, 1).astype(np.float32)

    nc = _get_nc()
    in_maps = []
    for i in range(NCORES):
        in_maps.append({
            "x": np.ascontiguousarray(x[i].reshape(C, H * W)),
            "wqk": wqk, "bqk": bqk, "wv": wv, "bv": bvr,
            "gamma": gam, "amat": amat, "bmat": bmat,
        })
    res = run_bass_kernel_spmd(nc, in_maps, core_ids=list(range(NCORES)))
    y = np.stack([r["y"].reshape(C, H, W) for r in res.results])
    return y.astype(np.float32)


if __name__ == "__main__":
    rng = np.random.default_rng(0)
    inputs = {
        "x": rng.standard_normal((B, C, H, W), dtype=np.float32),
        "Wq": rng.standard_normal((CQ, C), dtype=np.float32) * 0.05,
        "bq": rng.standard_normal((CQ,), dtype=np.float32) * 0.05,
        "Wk": rng.standard_normal((CQ, C), dtype=np.float32) * 0.05,
        "bk": rng.standard_normal((CQ,), dtype=np.float32) * 0.05,
        "Wv": rng.standard_normal((C, C), dtype=np.float32) * 0.05,
        "bv": rng.standard_normal((C,), dtype=np.float32) * 0.05,
        "gamma": np.zeros((1,), dtype=np.float32),
    }
    y = kernel(**inputs)
    print("out", y.shape, y.dtype, float(np.abs(y - inputs["x"]).max()))
